# revision 1
# baseline (speedup 1.0000x reference)
"""DMRG two-site effective Hamiltonian application (ApplyMPO) on 8 trn2 cores.

Math (reference):
  res[h,i,j,k] = sum_{a,b,c,d,e,f,g} L[b,h,a] M1[b,d,i,c] M2[d,f,j,e]
                                     R[f,k,g] psi[a,c,e,g]

Device algorithm (per core, output bond h sharded 8 x 128):
  Q[(b,c,e),(i,j,f)] = sum_d M1[b,d,i,c] M2[d,f,j,e]          (host, 400 els)
  step1: T1[h; (b,c,e), g]  = sum_a L[b,h,a] psi[a,(c,e),g]    (PE, K=a)
  flipA: T1P[(bce,g4); h]   = transpose of T1 4-g-column slices (PE transpose)
  mix:   T3H[h; (i,j,f),g4] = T1P^T @ Q4P  (Q4P = I4 (x) Q, zero-padded cols)
  flipB: T3G[g; (i,j,f), h] = transpose of T3H g-column slices  (PE transpose)
  step4: res[h; (i,j),k]   += T3G[:,ijf]^T @ R^T[f][g,k]        (PE, K=g, acc f)
All big matmuls run in float32r (fp32 storage, ~13-bit effective mantissa,
full-rate PE); transposes in fp32.
"""

import numpy as np

import concourse.bacc as bacc
import concourse.mybir as mybir
import concourse.tile as tile
from concourse import bass_utils

F32 = mybir.dt.float32
F32R = mybir.dt.float32r

CHI = 1024
W = 5
D = 2
NCORES = 8
H = CHI // NCORES  # 128, h rows per core

_nc_cache = None


def _build_nc():
    nc = bacc.Bacc("TRN2", target_bir_lowering=False)
    # host-prearranged: psi[ac, q, a_lo, ce, g256]; lt[b, a_lo, ac, h]; rt[blk, g_lo, f, k]
    psi = nc.dram_tensor("psi", [8, 4, 128, 4, 256], F32, kind="ExternalInput")
    lt = nc.dram_tensor("lt", [5, 128, 8, H], F32, kind="ExternalInput")
    rt = nc.dram_tensor("rt", [8, 128, 5, 1024], F32, kind="ExternalInput")
    q6 = nc.dram_tensor("q6", [120, 128], F32, kind="ExternalInput")
    q2 = nc.dram_tensor("q2", [40, 128], F32, kind="ExternalInput")
    idn = nc.dram_tensor("idn", [128, 128], F32, kind="ExternalInput")
    res = nc.dram_tensor("res", [H, 4096], F32, kind="ExternalOutput")  # h;(i,j,k)

    with tile.TileContext(nc) as tc:
        with (
            tc.tile_pool(name="const", bufs=1) as const_pool,
            tc.tile_pool(name="psis", bufs=8) as psi_pool,
            tc.tile_pool(name="t1", bufs=2) as t1_pool,
            tc.tile_pool(name="t1p", bufs=3) as t1p_pool,
            tc.tile_pool(name="t3h", bufs=2) as t3h_pool,
            tc.tile_pool(name="t3g", bufs=2) as t3g_pool,
            tc.tile_pool(name="rblk", bufs=2) as rblk_pool,
            tc.tile_pool(name="resp", bufs=1) as res_pool,
            tc.tile_pool(name="ps_s1", bufs=2, space="PSUM") as ps_s1,
            tc.tile_pool(name="ps_mid", bufs=2, space="PSUM") as ps_mid,
            tc.tile_pool(name="ps_s4", bufs=1, space="PSUM") as ps_s4,
        ):
            # ---- static loads (only b=0 weights gate the first matmul) ----
            lt_sb = const_pool.tile([128, 5, 8, H], F32R)  # [a_lo; b, ac, h]
            lt_r = lt.ap().bitcast(F32R).rearrange("b p ac h -> p b ac h")
            nc.sync.dma_start(lt_sb[:, 0], lt_r[:, 0])
            q6_sb = const_pool.tile([120, 128], F32R)
            q2_sb = const_pool.tile([40, 128], F32R)
            idn_sb = const_pool.tile([128, 128], F32R)
            res_sb = res_pool.tile([128, 4096], F32)

            def load_rest_of_consts():
                for b in range(1, 5):
                    nc.sync.dma_start(lt_sb[:, b], lt_r[:, b])
                nc.sync.dma_start(q6_sb[:], q6.ap().bitcast(F32R))
                nc.sync.dma_start(q2_sb[:], q2.ap().bitcast(F32R))
                nc.sync.dma_start(idn_sb[:], idn.ap().bitcast(F32R))

            evac_ct = 0

            def evac_copy(out, in_):
                # DVE is ~2x faster than ACT for copies; give ACT 1 in 3.
                nonlocal evac_ct
                evac_ct += 1
                if evac_ct % 3 == 0:
                    nc.scalar.copy(out, in_)
                else:
                    nc.vector.tensor_copy(out, in_)

            pending_s4_emitters = []

            for q in range(4):  # g-quarters
                # ---------- step 1: T1q[h; bce20, g256] ----------
                t1q = t1_pool.tile([128, 256, 20], F32R, tag="t1q")
                psis = []
                for ac in range(8):
                    pslice = psi_pool.tile([128, 4, 256], F32R, tag="psi")
                    nc.sync.dma_start(pslice[:], psi.ap().bitcast(F32R)[ac, q])
                    psis.append(pslice)
                if q == 0:
                    load_rest_of_consts()
                for b in range(5):
                    ps1 = ps_s1.tile([128, 4, 256], F32, tag="s1")  # 2 banks
                    ps1_flat = ps1[:].rearrange("p c g -> p (c g)")
                    for ac in range(8):
                        lhsT = lt_sb[:, b, ac]
                        psi_flat = psis[ac][:].rearrange("p c g -> p (c g)")
                        for cep in range(2):  # one 512-wide MM per PSUM bank
                            nc.tensor.matmul(
                                ps1_flat[:, cep * 512:(cep + 1) * 512],
                                lhsT,
                                psi_flat[:, cep * 512:(cep + 1) * 512],
                                start=(ac == 0),
                                stop=(ac == 7),
                            )
                    nc.vector.tensor_copy(
                        t1q[:, :, b * 4:b * 4 + 2],
                        ps1[:, 0:2].rearrange("p ce g -> p g ce"),
                    )
                    nc.scalar.copy(
                        t1q[:, :, b * 4 + 2:b * 4 + 4],
                        ps1[:, 2:4].rearrange("p ce g -> p g ce"),
                    )
                    if pending_s4_emitters and b < 4:
                        pending_s4_emitters[0][0](b)  # step4(q-1, ij=b)
                        if b == 3:
                            pending_s4_emitters.pop(0)

                t3gs = []
                rblks = []
                for blk2 in range(2):  # g-128 blocks in the quarter
                    blk = q * 2 + blk2
                    # ---------- flipA + mix: T3H[h; ijf20, g128] ----------
                    t3h = t3h_pool.tile([128, 20, 128], F32R, tag="t3h")
                    t1q_flat = t1q[:].rearrange("p g c -> p (g c)")

                    def flip_mix(goff, gsz, col, pa, t1p, pm):
                        sz = gsz * 20
                        base = (blk2 * 128 + goff) * 20
                        nc.tensor.transpose(
                            pa[0:sz, col * 128:(col + 1) * 128],
                            t1q_flat[:, base:base + sz],
                            idn_sb[:],
                        )
                        nc.tensor.matmul(
                            pm[:, col * 256:(col + 1) * 256],
                            t1p[0:sz, col * 128:(col + 1) * 128],
                            (q6_sb if gsz == 6 else q2_sb)[:],
                            start=True,
                            stop=True,
                        )

                    for pk in range(10):  # pack pairs: 2 x 6 g's
                        goff = pk * 12
                        pa = ps_mid.tile([128, 512], F32R, tag="mid")
                        t1p = t1p_pool.tile([120, 256], F32R, tag="t1p")
                        pm = ps_mid.tile([128, 512], F32, tag="mid")
                        nc.tensor.transpose(
                            pa[0:120, 0:128], t1q_flat[:, (blk2 * 128 + goff) * 20:(blk2 * 128 + goff) * 20 + 120], idn_sb[:]
                        )
                        nc.tensor.transpose(
                            pa[0:120, 128:256], t1q_flat[:, (blk2 * 128 + goff + 6) * 20:(blk2 * 128 + goff + 6) * 20 + 120], idn_sb[:]
                        )
                        evac_copy(t1p[:, 0:256], pa[0:120, 0:256])
                        for col in range(2):
                            nc.tensor.matmul(
                                pm[:, col * 128:(col + 1) * 128],
                                t1p[:, col * 128:(col + 1) * 128],
                                q6_sb[:],
                                start=True,
                                stop=True,
                            )
                        evac_copy(
                            t3h[:, :, goff:goff + 12].rearrange("p i (k g) -> p k i g", k=2),
                            pm[:, 0:256].rearrange("p (k r) -> p k r", k=2)[:, :, 0:120].rearrange(
                                "p k (i g) -> p k i g", g=6
                            ),
                        )
                    # ragged tail: one 6-pack + one 2-pack
                    for goff, gsz in [(120, 6), (126, 2)]:
                        sz = gsz * 20
                        pa = ps_mid.tile([128, 512], F32R, tag="mid")
                        t1p = t1p_pool.tile([120, 256], F32R, tag="t1p")
                        pm = ps_mid.tile([128, 512], F32, tag="mid")
                        base = (blk2 * 128 + goff) * 20
                        nc.tensor.transpose(pa[0:sz, 0:128], t1q_flat[:, base:base + sz], idn_sb[:])
                        evac_copy(t1p[0:sz, 0:128], pa[0:sz, 0:128])
                        nc.tensor.matmul(
                            pm[:, 0:128],
                            t1p[0:sz, 0:128],
                            (q6_sb if gsz == 6 else q2_sb)[:],
                            start=True,
                            stop=True,
                        )
                        evac_copy(
                            t3h[:, :, goff:goff + gsz],
                            pm[:, 0:sz].rearrange("p (i g) -> p i g", g=gsz),
                        )
                    # ---------- flipB: T3G[g; ijf20, h128] ----------
                    t3g = t3g_pool.tile([128, 20, 128], F32R, tag="t3g")
                    for ijq in range(5):  # 4 transposes per PSUM bank
                        pb = ps_mid.tile([128, 512], F32R, tag="mid")
                        for j in range(4):
                            nc.tensor.transpose(
                                pb[:, j * 128:(j + 1) * 128],
                                t3h[:, ijq * 4 + j, :],
                                idn_sb[:],
                            )
                        evac_copy(
                            t3g[:, ijq * 4:(ijq + 1) * 4, :].rearrange("p i h -> p (i h)"),
                            pb[:],
                        )
                    t3gs.append(t3g)
                    # R block load
                    rblk = rblk_pool.tile([128, 5, 1024], F32R, tag="rblk")
                    nc.sync.dma_start(rblk[:], rt.ap().bitcast(F32R)[blk])
                    rblks.append(rblk)

                # ---------- step 4: defer per-ij emitters; interleave with next q's step1
                def make_s4(qq, t3gs_, rblks_):
                    def emit_ij(ij):
                        ps4 = ps_s4.tile([128, 2, 512], F32, tag="s4")  # 2 banks
                        for blk2 in range(2):
                            for f in range(5):
                                lhsT = t3gs_[blk2][:, ij * 5 + f, :]
                                for kh in range(2):
                                    nc.tensor.matmul(
                                        ps4[:, kh],
                                        lhsT,
                                        rblks_[blk2][:, f, kh * 512:(kh + 1) * 512],
                                        start=(blk2 == 0 and f == 0),
                                        stop=(blk2 == 1 and f == 4),
                                    )
                        dst = res_sb[:, ij * 1024:(ij + 1) * 1024].rearrange(
                            "p (a b) -> p a b", b=512
                        )
                        if qq == 0:
                            evac_copy(dst, ps4[:])
                        else:
                            nc.vector.tensor_add(dst, dst, ps4[:])
                        if qq == 3:
                            nc.sync.dma_start(
                                res.ap()[:, ij * 1024:(ij + 1) * 1024],
                                res_sb[:, ij * 1024:(ij + 1) * 1024],
                            )

                    return emit_ij

                pending_s4_emitters.append((make_s4(q, t3gs, rblks), 4))

            # flush any remaining deferred step-4 work
            for emit, n in pending_s4_emitters:
                for ij in range(n):
                    emit(ij)
                pending_s4_emitters.clear()
                break
    nc.compile()
    return nc


def _host_inputs(psi_flat, L, M1, M2, R):
    # psi[a,ce,g] -> [ac, q, a_lo, ce, g256]
    psi = np.ascontiguousarray(
        psi_flat.reshape(8, 128, 4, 4, 256).transpose(0, 3, 1, 2, 4), dtype=np.float32
    )
    # R[f,k,g] -> RT[f,g,k] -> [blk, g_lo, f, k]
    RT = np.ascontiguousarray(
        R.transpose(2, 0, 1).reshape(8, 128, 5, 1024), dtype=np.float32
    )
    Q = np.einsum("bdic,dfje->bceijf", M1, M2).reshape(20, 20).astype(np.float32)
    rows = np.arange(20)
    Q6P = np.zeros((120, 128), np.float32)
    for g6 in range(6):
        Q6P[np.ix_(g6 * 20 + rows, rows * 6 + g6)] = Q
    Q2P = np.zeros((40, 128), np.float32)
    for g2 in range(2):
        Q2P[np.ix_(g2 * 20 + rows, rows * 2 + g2)] = Q
    idn = np.eye(128, dtype=np.float32)
    in_maps = []
    for c in range(NCORES):
        LT = np.ascontiguousarray(
            L[:, c * H:(c + 1) * H, :].transpose(0, 2, 1).reshape(5, 8, 128, H)
            .transpose(0, 2, 1, 3),
            dtype=np.float32,
        )  # [b, a_lo, ac, h]
        in_maps.append({"psi": psi, "lt": LT, "rt": RT, "q6": Q6P, "q2": Q2P, "idn": idn})
    return in_maps


def kernel(**inputs):
    psi_flat = np.asarray(inputs["psi_flat"], np.float32)
    L = np.asarray(inputs["L"], np.float32)
    M1 = np.asarray(inputs["M1"], np.float32)
    M2 = np.asarray(inputs["M2"], np.float32)
    R = np.asarray(inputs["R"], np.float32)

    global _nc_cache
    if _nc_cache is None:
        _nc_cache = _build_nc()
    nc = _nc_cache

    in_maps = _host_inputs(psi_flat, L, M1, M2, R)
    out = bass_utils.run_bass_kernel_spmd(nc, in_maps, core_ids=list(range(NCORES)))
    parts = [out.results[c]["res"] for c in range(NCORES)]
    return np.concatenate(parts, axis=0).reshape(-1)



# revision 7
# speedup vs baseline: 1.4405x; 1.4405x over previous
"""DMRG two-site effective Hamiltonian application (ApplyMPO) on 8 trn2 cores.

Math (reference):
  res[h,i,j,k] = sum_{a,b,c,d,e,f,g} L[b,h,a] M1[b,d,i,c] M2[d,f,j,e]
                                     R[f,k,g] psi[a,c,e,g]

Device algorithm (per core, output bond h sharded 8 x 128):
  Q[(b,c,e),(i,j,f)] = sum_d M1[b,d,i,c] M2[d,f,j,e]          (host, 400 els)
  step1: T1[h; (b,c,e), g]  = sum_a L[b,h,a] psi[a,(c,e),g]    (PE, K=a)
  flipA: T1P[(bce,g4); h]   = transpose of T1 4-g-column slices (PE transpose)
  mix:   T3H[h; (i,j,f),g4] = T1P^T @ Q4P  (Q4P = I4 (x) Q, zero-padded cols)
  flipB: T3G[g; (i,j,f), h] = transpose of T3H g-column slices  (PE transpose)
  step4: res[h; (i,j),k]   += T3G[:,ijf]^T @ R^T[f][g,k]        (PE, K=g, acc f)
All tensors in fp16 (1 cycle/row PE, exact fp32 PSUM accumulation);
transposes are exact fp16 byte moves.
"""

import numpy as np

import concourse.bacc as bacc
import concourse.mybir as mybir
import concourse.tile as tile
from concourse import bass_utils

F32 = mybir.dt.float32
FP16 = mybir.dt.float16

CHI = 1024
W = 5
D = 2
NCORES = 8
H = CHI // NCORES  # 128, h rows per core

_nc_cache = None


def _build_nc():
    nc = bacc.Bacc("TRN2", target_bir_lowering=False)
    # host-prearranged: psi[ac, q, a_lo, ce, g256]; lt[b, a_lo, ac, h]; rt[blk, g_lo, f, k]
    psi = nc.dram_tensor("psi", [8, 4, 128, 4, 256], FP16, kind="ExternalInput")
    lt = nc.dram_tensor("lt", [5, 128, 8, H], FP16, kind="ExternalInput")
    rt = nc.dram_tensor("rt", [8, 128, 5, 1024], FP16, kind="ExternalInput")
    q6 = nc.dram_tensor("q6", [120, 128], FP16, kind="ExternalInput")
    q2 = nc.dram_tensor("q2", [40, 128], FP16, kind="ExternalInput")
    idn = nc.dram_tensor("idn", [128, 128], FP16, kind="ExternalInput")
    res = nc.dram_tensor("res", [H, 4096], F32, kind="ExternalOutput")  # h;(i,j,k)

    with tile.TileContext(nc) as tc:
        with (
            tc.tile_pool(name="const", bufs=1) as const_pool,
            tc.tile_pool(name="psis", bufs=8) as psi_pool,
            tc.tile_pool(name="t1", bufs=2) as t1_pool,
            tc.tile_pool(name="t1p", bufs=3) as t1p_pool,
            tc.tile_pool(name="t3h", bufs=2) as t3h_pool,
            tc.tile_pool(name="t3g", bufs=2) as t3g_pool,
            tc.tile_pool(name="rblk", bufs=2) as rblk_pool,
            tc.tile_pool(name="resp", bufs=1) as res_pool,
            tc.tile_pool(name="ps_s1", bufs=2, space="PSUM") as ps_s1,
            tc.tile_pool(name="ps_mid", bufs=2, space="PSUM") as ps_mid,
            tc.tile_pool(name="ps_s4", bufs=1, space="PSUM") as ps_s4,
        ):
            # ---- static loads (only b=0 weights gate the first matmul) ----
            lt_sb = const_pool.tile([128, 5, 8, H], FP16)  # [a_lo; b, ac, h]
            lt_r = lt.ap().rearrange("b p ac h -> p b ac h")
            nc.sync.dma_start(lt_sb[:, 0], lt_r[:, 0])
            q6_sb = const_pool.tile([120, 128], FP16)
            q2_sb = const_pool.tile([40, 128], FP16)
            idn_sb = const_pool.tile([128, 128], FP16)
            res_sb = res_pool.tile([128, 4096], F32)

            def load_rest_of_consts():
                for b in range(1, 5):
                    nc.sync.dma_start(lt_sb[:, b], lt_r[:, b])
                nc.sync.dma_start(q6_sb[:], q6.ap())
                nc.sync.dma_start(q2_sb[:], q2.ap())
                nc.sync.dma_start(idn_sb[:], idn.ap())

            evac_ct = 0

            def evac_copy(out, in_):
                # only DVE and ACT can read PSUM; alternate them 1:1
                nonlocal evac_ct
                evac_ct += 1
                if evac_ct % 2 == 0:
                    nc.scalar.copy(out, in_)
                else:
                    nc.vector.tensor_copy(out, in_)

            pending_s4_emitters = []

            for q in range(4):  # g-quarters
                # ---------- step 1: T1q[h; bce20, g256] ----------
                t1q = t1_pool.tile([128, 256, 20], FP16, tag="t1q")
                psis = []
                for ac in range(8):
                    pslice = psi_pool.tile([128, 4, 256], FP16, tag="psi")
                    nc.sync.dma_start(pslice[:], psi.ap()[ac, q])
                    psis.append(pslice)
                if q == 0:
                    load_rest_of_consts()
                for b in range(5):
                    ps1 = ps_s1.tile([128, 4, 256], F32, tag="s1")  # 2 banks
                    ps1_flat = ps1[:].rearrange("p c g -> p (c g)")
                    for ac in range(8):
                        lhsT = lt_sb[:, b, ac]
                        psi_flat = psis[ac][:].rearrange("p c g -> p (c g)")
                        for cep in range(2):  # one 512-wide MM per PSUM bank
                            nc.tensor.matmul(
                                ps1_flat[:, cep * 512:(cep + 1) * 512],
                                lhsT,
                                psi_flat[:, cep * 512:(cep + 1) * 512],
                                start=(ac == 0),
                                stop=(ac == 7),
                            )
                    nc.vector.tensor_copy(
                        t1q[:, :, b * 4:b * 4 + 2],
                        ps1[:, 0:2].rearrange("p ce g -> p g ce"),
                    )
                    nc.scalar.copy(
                        t1q[:, :, b * 4 + 2:b * 4 + 4],
                        ps1[:, 2:4].rearrange("p ce g -> p g ce"),
                    )
                    if pending_s4_emitters and b < 4:
                        pending_s4_emitters[0][0](b)  # step4(q-1, ij=b)
                        if b == 3:
                            pending_s4_emitters.pop(0)

                t3gs = []
                rblks = []
                for blk2 in range(2):  # g-128 blocks in the quarter
                    blk = q * 2 + blk2
                    # ---------- flipA + mix: T3H[h; ijf20, g128] ----------
                    t3h = t3h_pool.tile([128, 20, 128], FP16, tag="t3h")
                    t1q_flat = t1q[:].rearrange("p g c -> p (g c)")

                    for pk in range(10):  # pack pairs: 2 x 6 g's
                        goff = pk * 12
                        pa32 = ps_mid.tile([128, 512], F32, tag="mid")
                        pa = pa32[:].bitcast(FP16)  # [128, 1024] fp16 view
                        t1p = t1p_pool.tile([120, 256], FP16, tag="t1p")
                        pm = ps_mid.tile([128, 512], F32, tag="mid")
                        nc.tensor.transpose(
                            pa[0:120, 0:128], t1q_flat[:, (blk2 * 128 + goff) * 20:(blk2 * 128 + goff) * 20 + 120], idn_sb[:]
                        )
                        nc.tensor.transpose(
                            pa[0:120, 128:256], t1q_flat[:, (blk2 * 128 + goff + 6) * 20:(blk2 * 128 + goff + 6) * 20 + 120], idn_sb[:]
                        )
                        evac_copy(t1p[:, 0:256], pa[0:120, 0:256])
                        for col in range(2):
                            nc.tensor.matmul(
                                pm[:, col * 128:(col + 1) * 128],
                                t1p[:, col * 128:(col + 1) * 128],
                                q6_sb[:],
                                start=True,
                                stop=True,
                            )
                        evac_copy(
                            t3h[:, :, goff:goff + 12].rearrange("p i (k g) -> p k i g", k=2),
                            pm[:, 0:256].rearrange("p (k r) -> p k r", k=2)[:, :, 0:120].rearrange(
                                "p k (i g) -> p k i g", g=6
                            ),
                        )
                    # ragged tail: one 6-pack + one 2-pack
                    for goff, gsz in [(120, 6), (126, 2)]:
                        sz = gsz * 20
                        pa32 = ps_mid.tile([128, 512], F32, tag="mid")
                        pa = pa32[:].bitcast(FP16)
                        t1p = t1p_pool.tile([120, 256], FP16, tag="t1p")
                        pm = ps_mid.tile([128, 512], F32, tag="mid")
                        base = (blk2 * 128 + goff) * 20
                        nc.tensor.transpose(pa[0:sz, 0:128], t1q_flat[:, base:base + sz], idn_sb[:])
                        evac_copy(t1p[0:sz, 0:128], pa[0:sz, 0:128])
                        nc.tensor.matmul(
                            pm[:, 0:128],
                            t1p[0:sz, 0:128],
                            (q6_sb if gsz == 6 else q2_sb)[:],
                            start=True,
                            stop=True,
                        )
                        evac_copy(
                            t3h[:, :, goff:goff + gsz],
                            pm[:, 0:sz].rearrange("p (i g) -> p i g", g=gsz),
                        )
                    # ---------- flipB: T3G[g; ijf20, h128] ----------
                    t3g = t3g_pool.tile([128, 20, 128], FP16, tag="t3g")
                    for ijq in range(5):  # 4 fp16 transposes per half PSUM bank
                        pb32 = ps_mid.tile([128, 512], F32, tag="mid")
                        pb = pb32[:].bitcast(FP16)
                        for j in range(4):
                            nc.tensor.transpose(
                                pb[:, j * 128:(j + 1) * 128],
                                t3h[:, ijq * 4 + j, :],
                                idn_sb[:],
                            )
                        evac_copy(
                            t3g[:, ijq * 4:(ijq + 1) * 4, :].rearrange("p i h -> p (i h)"),
                            pb[:, 0:512],
                        )
                    t3gs.append(t3g)
                    # R block load
                    rblk = rblk_pool.tile([128, 5, 1024], FP16, tag="rblk")
                    nc.sync.dma_start(rblk[:], rt.ap()[blk])
                    rblks.append(rblk)

                # ---------- step 4: defer per-ij emitters; interleave with next q's step1
                def make_s4(qq, t3gs_, rblks_):
                    def emit_ij(ij):
                        ps4 = ps_s4.tile([128, 2, 512], F32, tag="s4")  # 2 banks
                        for blk2 in range(2):
                            for f in range(5):
                                lhsT = t3gs_[blk2][:, ij * 5 + f, :]
                                for kh in range(2):
                                    nc.tensor.matmul(
                                        ps4[:, kh],
                                        lhsT,
                                        rblks_[blk2][:, f, kh * 512:(kh + 1) * 512],
                                        start=(blk2 == 0 and f == 0),
                                        stop=(blk2 == 1 and f == 4),
                                    )
                        dst = res_sb[:, ij * 1024:(ij + 1) * 1024].rearrange(
                            "p (a b) -> p a b", b=512
                        )
                        if qq == 0:
                            evac_copy(dst, ps4[:])
                        else:
                            nc.vector.tensor_add(dst, dst, ps4[:])
                        if qq == 3:
                            nc.sync.dma_start(
                                res.ap()[:, ij * 1024:(ij + 1) * 1024],
                                res_sb[:, ij * 1024:(ij + 1) * 1024],
                            )

                    return emit_ij

                pending_s4_emitters.append((make_s4(q, t3gs, rblks), 4))

            # flush any remaining deferred step-4 work
            for emit, n in pending_s4_emitters:
                for ij in range(n):
                    emit(ij)
                pending_s4_emitters.clear()
                break
    nc.compile()
    return nc


def _host_inputs(psi_flat, L, M1, M2, R):
    # psi[a,ce,g] -> [ac, q, a_lo, ce, g256]
    psi = np.ascontiguousarray(
        psi_flat.reshape(8, 128, 4, 4, 256).transpose(0, 3, 1, 2, 4)
    ).astype(np.float16)
    # R[f,k,g] -> RT[f,g,k] -> [blk, g_lo, f, k]
    RT = np.ascontiguousarray(
        R.transpose(2, 0, 1).reshape(8, 128, 5, 1024)
    ).astype(np.float16)
    Q = np.einsum("bdic,dfje->bceijf", M1, M2).reshape(20, 20).astype(np.float32)
    rows = np.arange(20)
    Q6P = np.zeros((120, 128), np.float32)
    for g6 in range(6):
        Q6P[np.ix_(g6 * 20 + rows, rows * 6 + g6)] = Q
    Q2P = np.zeros((40, 128), np.float32)
    for g2 in range(2):
        Q2P[np.ix_(g2 * 20 + rows, rows * 2 + g2)] = Q
    idn = np.eye(128, dtype=np.float16)
    q6h = Q6P.astype(np.float16)
    q2h = Q2P.astype(np.float16)
    in_maps = []
    for c in range(NCORES):
        LT = np.ascontiguousarray(
            L[:, c * H:(c + 1) * H, :].transpose(0, 2, 1).reshape(5, 8, 128, H)
            .transpose(0, 2, 1, 3)
        ).astype(np.float16)  # [b, a_lo, ac, h]
        in_maps.append({"psi": psi, "lt": LT, "rt": RT, "q6": q6h, "q2": q2h, "idn": idn})
    return in_maps


def kernel(**inputs):
    psi_flat = np.asarray(inputs["psi_flat"], np.float32)
    L = np.asarray(inputs["L"], np.float32)
    M1 = np.asarray(inputs["M1"], np.float32)
    M2 = np.asarray(inputs["M2"], np.float32)
    R = np.asarray(inputs["R"], np.float32)

    global _nc_cache
    if _nc_cache is None:
        _nc_cache = _build_nc()
    nc = _nc_cache

    in_maps = _host_inputs(psi_flat, L, M1, M2, R)
    out = bass_utils.run_bass_kernel_spmd(nc, in_maps, core_ids=list(range(NCORES)))
    parts = [out.results[c]["res"] for c in range(NCORES)]
    return np.concatenate(parts, axis=0).reshape(-1)
